# revision 8
# baseline (speedup 1.0000x reference)
"""Bhattacharyya coefficient kernel for Trainium2 (8 NeuronCores, SPMD).

out[n,0,i,j] = (1/k^2) * sum_{c,p,q} w[c] * sqrt(x[n,c,i+p,j+q] * z[n,c,p,q])

Data-parallel over batch: 2 samples per core. Per sample:
  1. ACT: sx = sqrt(x) (bf16), szw = w/k^2 * sqrt(z) (bf16).
  2. TensorE: plane[t, y] = sum_c szw[c, t] * sx[c, y] for the 64 taps
     t = 8p+q and all 63*63 image pixels y (K=256 in two 128-chunks
     accumulated in PSUM, M=64 taps, N in 8 blocks of <=512).
  3. Evict PSUM -> SBUF (bf16), dump plane to a DRAM scratch.
  4. Gather from DRAM with per-tap shifted offsets (flat DRAM AP):
     A[t, u] = plane[t, u + 63*(t>>3) + (t&7)], so the tap-sum becomes a
     pure partition reduction.
  5. TensorE ones-matmul: o[u] = sum_t A[t, u]; evict; out[i,j] = o[63i+j].
"""

import numpy as np

import concourse.bacc as bacc
import concourse.bass as bass
import concourse.mybir as mybir
from concourse import tile
from concourse.bass_utils import run_bass_kernel_spmd

N, C, KS, MS = 16, 256, 8, 63
MO = MS - KS + 1            # 56
F = MS * MS                 # 3969
L = (MO - 1) * MS + MO + 2  # 3522 (even; last needed flat index is 63*55+55)
NCORES = 8
SPC = N // NCORES           # samples per core
BLK = 512
NBLK = (F + BLK - 1) // BLK  # 8
NBLK2 = (L + BLK - 1) // BLK  # 7
AF = mybir.ActivationFunctionType
f32 = mybir.dt.float32
bf16 = mybir.dt.bfloat16

_CACHE = {}


def _build():
    nc = bacc.Bacc("TRN2", target_bir_lowering=False, debug=False)
    z_in = nc.declare_dram_parameter("z", [SPC, C, KS, KS], f32, isOutput=False)
    x_in = nc.declare_dram_parameter("x", [SPC, C, MS, MS], f32, isOutput=False)
    w_in = nc.declare_dram_parameter("w", [C], f32, isOutput=False)
    out = nc.declare_dram_parameter("out", [SPC, 1, MO, MO], f32, isOutput=True)
    FP = F + 8  # padded row pitch in the DRAM scratch (AP slack)
    pl_dram = nc.dram_tensor("pl_scratch", [SPC, 64, FP], bf16)

    with tile.TileContext(nc) as tc:
        with (
            tc.tile_pool(name="xstage", bufs=3) as xstage,
            tc.tile_pool(name="sxq", bufs=2) as sxq,
            tc.tile_pool(name="zpool", bufs=2) as zpool,
            tc.tile_pool(name="plane", bufs=2) as plane,
            tc.tile_pool(name="gath", bufs=1) as gath,
            tc.tile_pool(name="opool", bufs=2) as opool,
            tc.tile_pool(name="psum", bufs=4, space="PSUM") as psum,
            tc.tile_pool(name="psum2", bufs=4, space="PSUM") as psum2,
        ):
            # all-ones [128, 1] bf16 for the tap-reduction matmul
            ones = opool.tile([128, 1], bf16, name="ones")
            nc.gpsimd.memset(ones[:], 1.0)
            a2 = gath.tile([128, L], bf16, name="a2")

            for s in range(SPC):
                obuf = opool.tile([1, 3584], f32, tag=f"ob{s}", name=f"obuf{s}")
                # ---- z path: szw[c, k, t] = w[c]/64 * sqrt(z[c, t]) ----
                zt = zpool.tile([128, 2, KS * KS], f32, tag="zt", name=f"zt{s}")
                nc.sync.dma_start(
                    zt[:], z_in[s].rearrange("(k c) p q -> c k (p q)", c=128)
                )
                wt = zpool.tile([128, 2], f32, tag="wt", name=f"wt{s}")
                nc.sync.dma_start(wt[:], w_in.rearrange("(k c) -> c k", c=128))
                w64 = zpool.tile([128, 2], f32, tag="w64", name=f"w64{s}")
                nc.scalar.mul(w64[:], wt[:], 1.0 / (KS * KS))
                zsq = zpool.tile([128, 2, KS * KS], f32, tag="zsq", name=f"zsq{s}")
                szw = zpool.tile([128, 2, KS * KS], bf16, tag="szw", name=f"szw{s}")
                for k in range(2):
                    nc.scalar.activation(zsq[:, k, :], zt[:, k, :], AF.Sqrt)
                    nc.vector.tensor_scalar_mul(
                        szw[:, k, :], zsq[:, k, :], w64[:, k : k + 1]
                    )

                # ---- x path: sx[c, k, y] = sqrt(x[c_global, y]) (bf16) ----
                sx = sxq.tile([128, 2, F], bf16, tag="sx", name=f"sx{s}")
                for k in range(2):
                    xst = xstage.tile([128, F], f32, tag="xst", name=f"xst{s}{k}")
                    nc.sync.dma_start(
                        xst[:],
                        x_in[s, k * 128 : (k + 1) * 128].rearrange("c h w -> c (h w)"),
                    )
                    nc.scalar.activation(sx[:, k, :], xst[:], AF.Sqrt)

                # ---- matmuls: plane[t, y] = sum_c szw[c, t] sx[c, y] ----
                pl = plane.tile([64, F], bf16, tag="pl", name=f"pl{s}")
                for b in range(NBLK):
                    nb = min(BLK, F - b * BLK)
                    ps = psum.tile([64, BLK], f32, tag="ps", name=f"ps_{s}_{b}")
                    for k in range(2):
                        nc.tensor.matmul(
                            ps[:, :nb],
                            szw[:, k, :],
                            sx[:, k, b * BLK : b * BLK + nb],
                            start=(k == 0),
                            stop=(k == 1),
                        )
                    # evict (cast bf16)
                    dst = pl[:, b * BLK : b * BLK + nb]
                    if b % 2 == 0:
                        nc.scalar.copy(dst, ps[:, :nb])
                    else:
                        nc.vector.tensor_copy(dst, ps[:, :nb])
                nc.sync.dma_start(pl_dram[s, :, 0:F], pl[:])

                # ---- gather with per-tap shift: A[t, u] = plane[t, u+off(t)]
                src = bass.AP(
                    pl_dram[:].tensor,
                    s * 64 * FP,
                    [[8 * FP + MS, 8], [FP + 1, 8], [1, L]],
                )
                nc.sync.dma_start(a2[64 * s : 64 * s + 64, :], src)

                # ---- tap reduction: o[u] = sum_t A[t, u] ----
                for b in range(NBLK2):
                    nb = min(BLK, L - b * BLK)
                    ps2 = psum2.tile([1, BLK], f32, tag="ps2", name=f"ps2_{s}_{b}")
                    nc.tensor.matmul(
                        ps2[:, :nb],
                        ones[64 * s : 64 * s + 64, :],
                        a2[64 * s : 64 * s + 64, b * BLK : b * BLK + nb],
                        start=True,
                        stop=True,
                    )
                    nc.vector.tensor_copy(obuf[0:1, b * BLK : b * BLK + nb], ps2[:, :nb])

                # ---- extract valid rows: out[i, j] = o[63 i + j] ----
                osrc = obuf[0:1, 0 : MO * MS].rearrange("p (i j) -> p i j", i=MO)[
                    :, :, 0:MO
                ]
                nc.sync.dma_start(out[s, 0].unsqueeze(0), osrc)

    nc.compile()
    return nc


def _get_nc():
    if "nc" not in _CACHE:
        _CACHE["nc"] = _build()
    return _CACHE["nc"]


def _run(z, x, weights, **runkw):
    z = np.ascontiguousarray(np.asarray(z), dtype=np.float32)
    x = np.ascontiguousarray(np.asarray(x), dtype=np.float32)
    w = np.ascontiguousarray(np.asarray(weights), dtype=np.float32).reshape(C)
    in_maps = []
    for i in range(NCORES):
        lo, hi = i * SPC, (i + 1) * SPC
        in_maps.append({"z": z[lo:hi], "x": x[lo:hi], "w": w})
    nc = _get_nc()
    res = run_bass_kernel_spmd(nc, in_maps, core_ids=list(range(NCORES)), **runkw)
    full = np.concatenate([res.results[i]["out"] for i in range(NCORES)], axis=0)
    return full, res


def kernel(z, x, weights):
    full, _ = _run(z, x, weights)
    return full
